# revision 1
# baseline (speedup 1.0000x reference)
"""Trainium2 Bass kernel for nn_CapsuleLayer (capsule layer: einsum + squash).

  u_hat = einsum('croi,bri->bcro', W[0], x)   # x:[256,1152,8] W:[1,10,1152,16,8]
  out   = squash(u_hat)                       # squash over last (o) axis

Strategy (8 NeuronCores, routes sharded 144/core, full batch per core):
  - Groups of 3 routes; 4 groups per quad-block q; per (q, half):
      u-MM:  stationary x^T strip [32=(3r x 8i + pad), 128 batch], moving
             block-diagonal W [32, 480] -> per-group 2-bank psum tile
             [128, 1024] (batch-half h at column 512h).
      sq-MM: stationary xx pair-products [128=(3 x 36 pairs + pad), 128 b],
             moving block-diag sym-Gram cols [128, 30] -> a DEDICATED psum
             bank [128, 120=(group j, h, 30)].
    where xx[b,(i,j)] = x_i*x_j (i<=j) and Gsym[(i,j),c] = (2-delta_ij)*G[i,j]
    with G = W_cr^T W_cr, so the PE emits sq_norm[b,(r,c)] = ||u||^2 directly.
    sq owning its own bank matters: PE-write + ACT-read of one psum bank is
    a fatal collision, so if sq shared the u banks the scale chain could not
    start until ALL matmuls of the half-block finished; with a dedicated
    bank (and sq-MMs issued first) the chain overlaps the u-MM streaming,
    and the steady state is purely DVE-rate-limited (~93% DVE duty).
  - PSUM budget: 3 x 2-bank u tiles + 2 x 1-bank sq tiles = 8 banks.
  - squash scale s = sq/((1+sq)*sqrt(sq+1e-9)) = sqrt(sq) * 1/(1+sq)
    = Exp(0.5*Ln(sq)) * Exp(-Ln(1+sq)): ACT reads sq straight from the
    dedicated psum bank (Ln x2, Exp x2, one act-table set) and the combine
    multiply runs on the otherwise-idle GPSIMD (SBUF-only tensor_tensor is
    Pool-legal), keeping the DVE stream to exactly the big multiplies.
  - DVE: one strided multiply per group ([128,(2h,30rc,16o)] from psum,
    s broadcast over o) -> dense fp16 [128, 960] halves of an out tile,
    software-pipelined one half-block behind the scale chain; its store
    (one [2h,128,480] DMA per group on the sync HWDGE) issues right after.
  - Everything is fp16 end-to-end (inputs, matmuls, output): same bytes and
    engine perf-modes as bf16 but 11 mantissa bits, and every tensor here
    is small (|xx|<~25, |u|<~50, |out|<1).  PSUM/scale math stays fp32.
    Measured rel err ~1.3e-3 vs the fp32 reference.
  - Startup-critical DMA ordering: xx(q0)+gs(q0)+xs(q0) first on the sync
    HWDGE FIFO, bulk xs/gs after, wm on the ACT HWDGE ring, steady-state
    xx on the gpsimd SWDGE queue -- the first sq-MM fires after ~0.4 MB.
"""

import sys

if "/opt/trn_rl_repo" not in sys.path:
    sys.path.insert(0, "/opt/trn_rl_repo")

from contextlib import ExitStack

import numpy as np

import concourse.bacc as bacc
import concourse.bass as bass
import concourse.mybir as mybir
import concourse.tile as tile
from concourse._compat import with_exitstack
from concourse.bass_utils import run_bass_kernel_spmd

# Problem shapes (hardcoded; harness provides full inputs)
B = 256          # batch
R = 1152         # num routes
C = 10           # num capsules
O = 16           # out channels
I = 8            # in channels
NCORES = 8
RL = R // NCORES                 # 144 routes per core
NG = RL // 3                     # 48 groups of 3 routes
NQ = NG // 4                     # 12 quad-blocks of 4 groups
NPAIR = 36                       # i<=j pairs of 8 inputs
F32 = mybir.dt.float32
BF16 = mybir.dt.bfloat16
F16 = mybir.dt.float16
PAIRS = [(i, j) for i in range(I) for j in range(i, I)]


def _dtypes(mode: str):
    """(u-path dtype, sq-path dtype, output dtype) for a mode string.

    Default is fp16 end-to-end: same byte-width (and thus DMA cost and DVE
    perf-mode) as bf16, but 11 mantissa bits instead of 8 — and every tensor
    here is small (|xx|<~25, |u|<~50, |out|<1), far inside fp16 range."""
    if "allf32r" in mode:
        u_dt = sq_dt = mybir.dt.float32r
    elif "bf16" in mode:
        u_dt = sq_dt = BF16
    else:
        u_dt = sq_dt = F16
    out_dt = F32 if "f32out" in mode else (BF16 if "bf16" in mode else F16)
    return u_dt, sq_dt, out_dt


@with_exitstack
def _capsule_body(ctx: ExitStack, tc: "tile.TileContext",
                  out: bass.AP, xs: bass.AP, wm: bass.AP,
                  xxs: bass.AP, gs: bass.AP, reps: int = 1,
                  mode: str = "full"):
    nc = tc.nc
    u_dt, sq_dt, out_dt = _dtypes(mode)

    singles = ctx.enter_context(tc.tile_pool(name="singles", bufs=1))
    wm_pool = ctx.enter_context(tc.tile_pool(name="wm", bufs=3))
    xx_pool = ctx.enter_context(tc.tile_pool(name="xx", bufs=3))
    # PSUM: u tiles are 2 banks (one group x two batch-halves) x 3 bufs,
    # sq gets its OWN bank (x2) so the scale chain never waits on (or
    # collides with) the u matmuls' banks.
    u_pool = ctx.enter_context(tc.tile_pool(name="upsum", bufs=3, space="PSUM"))
    sq_pool = ctx.enter_context(tc.tile_pool(name="sqpsum", bufs=2, space="PSUM"))
    smalls = ctx.enter_context(tc.tile_pool(name="smalls", bufs=3))
    out_pool = ctx.enter_context(tc.tile_pool(name="outs", bufs=8))

    # Startup-critical loads first on the sync HWDGE FIFO: q=0's xx + gram
    # slice (first sq-MMs), then q=0's x slice (first u-MMs); the bulk of
    # xs/gs follows as two big DMAs.
    xx0_t = singles.tile([128, 4 * B], sq_dt)
    nc.sync.dma_start(out=xx0_t[:], in_=xxs[0].rearrange("p k b -> p (k b)"))
    xs_sb = singles.tile([128, NQ * B], u_dt)
    gs_sb = singles.tile([128, NG * 30], sq_dt)
    nc.sync.dma_start(out=gs_sb[:, 0:120],
                      in_=gs[:, 0:4].rearrange("p g n -> p (g n)"))
    nc.sync.dma_start(out=xs_sb[:, 0:B], in_=xs[:, 0])
    nc.sync.dma_start(out=gs_sb[:, 120:],
                      in_=gs[:, 4:].rearrange("p g n -> p (g n)"))
    nc.sync.dma_start(out=xs_sb[:, B:],
                      in_=xs[:, 1:].rearrange("p q b -> p (q b)"))

    if reps > 1:
        # Timing-only variant: run the whole body `reps` times on-device so
        # wall-clock differences cancel host/axon overhead.
        loop_cm = tc.For_i(0, reps, 1)
        ctx.enter_context(loop_cm)

    def emit_muls(pend):
        g0 = 4 * pend[-2] + 2 * pend[-1]
        ot = out_pool.tile([128, 1920], out_dt, tag="ot")
        U0, U1, s_t = pend[0], pend[1], pend[2]
        for j, U in ((0, U0), (1, U1)):
            u_ap = (U[:].rearrange("p (h r v) -> p h r v", h=2, r=32)
                    [:, :, 0:30, :])
            s_ap = (s_t[:, 60 * j: 60 * j + 60]
                    .rearrange("p (h r) -> p h r", h=2)
                    .unsqueeze(3).broadcast_to([128, 2, 30, O]))
            nc.vector.tensor_mul(
                ot[:, 960 * j: 960 * j + 960]
                .rearrange("p (h r v) -> p h r v", h=2, r=30), u_ap, s_ap)
            if "noout" not in mode:
                # Per-group DMA right after its mul: the last store overlaps
                # the sibling mul instead of trailing both.
                nc.sync.dma_start(
                    out=out[:, g0 + j].rearrange("h p v -> p h v"),
                    in_=ot[:, 960 * j: 960 * j + 960]
                    .rearrange("p (h v) -> p h v", h=2))

    pending = None
    for q in range(NQ):
        wm_t = wm_pool.tile([128, 480], u_dt)
        nc.scalar.dma_start(out=wm_t[:], in_=wm[q])
        if q == 0:
            xx_t = xx0_t        # preloaded on the sync queue ahead of bulk
        else:
            xx_t = xx_pool.tile([128, 4 * B], sq_dt)
            nc.gpsimd.dma_start(out=xx_t[:],
                                in_=xxs[q].rearrange("p k b -> p (k b)"))
        for half in range(2):
            u0_t = u_pool.tile([128, 1024], F32, tag="u")
            u1_t = u_pool.tile([128, 1024], F32, tag="u")
            Us = [u0_t, u1_t]
            S = sq_pool.tile([128, 120], F32, tag="sq")
            # sq matmuls FIRST (own bank): the scale chain kicks off while
            # the u matmuls still stream.
            for j in range(2):
                gk = 2 * half + j
                g = 4 * q + gk
                for h in range(2):
                    nc.tensor.matmul(
                        S[:, 60 * j + 30 * h: 60 * j + 30 * h + 30],
                        xx_t[:, gk * B + h * 128: gk * B + h * 128 + 128],
                        gs_sb[:, g * 30: g * 30 + 30], start=True, stop=True,
                        tile_position=(0, 0))
            for j in range(2):
                gk = 2 * half + j
                for h in range(2):
                    nc.tensor.matmul(
                        Us[j][:, 512 * h: 512 * h + 480],
                        xs_sb[32 * gk: 32 * gk + 32,
                              q * B + h * 128: q * B + h * 128 + 128],
                        wm_t[32 * gk: 32 * gk + 32, :], start=True, stop=True,
                        tile_position=(32 * gk, 0))

            if "nosquash" in mode:
                continue

            # Scale chain from the dedicated sq bank (dense [128,120]):
            # s = sqrt(sq)/(1+sq) = Exp(0.5*Ln(sq)) * Exp(-Ln(1+sq)).
            # The combine runs on GPSIMD (SBUF-only tensor_tensor is
            # Pool-legal) so DVE's stream is purely the big multiplies.
            lnsq = smalls.tile([128, 120], F32, tag="lnsq")
            nc.scalar.activation(lnsq[:], S[:],
                                 mybir.ActivationFunctionType.Ln)
            ln1p = smalls.tile([128, 120], F32, tag="ln1p")
            nc.scalar.activation(ln1p[:], S[:],
                                 mybir.ActivationFunctionType.Ln, bias=1.0)
            rtsq = smalls.tile([128, 120], F32, tag="rtsq")
            nc.scalar.activation(rtsq[:], lnsq[:],
                                 mybir.ActivationFunctionType.Exp, scale=0.5)
            rcp1 = smalls.tile([128, 120], F32, tag="rcp1")
            nc.scalar.activation(rcp1[:], ln1p[:],
                                 mybir.ActivationFunctionType.Exp, scale=-1.0)
            s_t = smalls.tile([128, 120], F32, tag="s")
            nc.gpsimd.tensor_tensor(out=s_t[:], in0=rtsq[:], in1=rcp1[:],
                                    op=mybir.AluOpType.mult)

            if pending is not None:
                emit_muls(pending)
            pending = (Us[0], Us[1], s_t, q, half)

    if pending is not None:
        emit_muls(pending)


def build_bass(reps: int = 1, mode: str = "full"):
    # Bacc (not plain Bass): its compile() runs generate_event_semaphores,
    # which splits multi-semaphore waits — TPB instructions carry only one
    # wait slot in hardware — plus move_matmul_waits_to_ldweights etc.
    nc = bacc.Bacc("TRN2", target_bir_lowering=False, debug=False,
                   num_devices=NCORES)
    u_dt, sq_dt, out_dt = _dtypes(mode)
    xs = nc.dram_tensor("xs", [128, NQ, B], u_dt, kind="ExternalInput")
    wm = nc.dram_tensor("wm", [NQ, 128, 480], u_dt, kind="ExternalInput")
    xxs = nc.dram_tensor("xxs", [NQ, 128, 4, B], sq_dt, kind="ExternalInput")
    gs = nc.dram_tensor("gs", [128, NG, 30], sq_dt, kind="ExternalInput")
    out = nc.dram_tensor("out", [2, NG, 128, 480], out_dt,
                         kind="ExternalOutput")
    with tile.TileContext(nc) as tc:
        _capsule_body(tc, out[:], xs[:], wm[:], xxs[:], gs[:],
                      reps=reps, mode=mode)

    # All ACT functions used here (Ln, Exp) coexist in the
    # natural_log_exp_and_others table set, but the stock table-load pass
    # assigns each function its *first* containing set, alternating sets and
    # inserting ~2.7us table loads throughout.  Strip our functions from all
    # other sets (keeping positional act_func_set ids intact) so resolution
    # lands on the one set and a single load is emitted.
    import types
    from concourse.hw_specs import get_activation_tables
    from concourse import bacc as _bacc_mod

    _PIN = "natural_log_exp_and_others"
    _FUNCS = {mybir.ActivationFunctionType.Square,
              mybir.ActivationFunctionType.Ln,
              mybir.ActivationFunctionType.Exp,
              mybir.ActivationFunctionType.Copy,
              mybir.ActivationFunctionType.Identity}

    def _one_set_table_loads(self):
        tables = [
            (k, (v if k == _PIN else (v - _FUNCS)))
            for k, v in get_activation_tables(self.m.arch).items()
        ]
        _bacc_mod._bass_rust.insert_act_table_loads(self, tables)

    nc.insert_act_table_loads = types.MethodType(_one_set_table_loads, nc)
    nc.compile()
    return nc


_NC = {}


def _get_nc(reps: int = 1, mode: str = "full"):
    key = (reps, mode)
    if key not in _NC:
        _NC[key] = build_bass(reps, mode)
    return _NC[key]


def _pack_inputs(x: np.ndarray, W: np.ndarray):
    """Build per-core xs [128,12,256], wm [12,128,480], xxs [12,128,4,256],
    gs [128,48,30] (fp32; cast per-mode at dispatch)."""
    x = np.ascontiguousarray(x, dtype=np.float32)
    W0 = np.ascontiguousarray(W.reshape(C, R, O, I), dtype=np.float32)

    # x stationaries: [R, I, B] -> rows padded to 32, 4 groups stacked on the
    # 128 partitions (full-width DMA): [cores, 128=(k,row), NQ, B]
    xt = x.transpose(1, 2, 0)                        # [R, I, B]
    xs = np.zeros((NCORES, NG, 32, B), np.float32)
    xs[:, :, :24] = xt.reshape(NCORES, NG, 24, B)
    xs = xs.reshape(NCORES, NQ, 4, 32, B).transpose(0, 2, 3, 1, 4)
    xs = np.ascontiguousarray(xs.reshape(NCORES, 128, NQ, B))

    # W moving blocks, 4 groups stacked on partitions: [cores, NQ, 128, 480]
    Wt = W0.transpose(1, 3, 0, 2)                    # [R, I, C, O]
    Wt = Wt.reshape(NCORES, NG, 3, I, C * O)         # k,g,r,i,co
    wm = np.zeros((NCORES, NG, 32, 3, C * O), np.float32)
    for r in range(3):
        wm[:, :, r * I:(r + 1) * I, r] = Wt[:, :, r]
    wm = np.ascontiguousarray(wm.reshape(NCORES, NQ, 128, 480))

    # xx pair products: [B, R, 36] -> [cores, NQ, 128, 4, B]
    ii = np.array([p[0] for p in PAIRS])
    jj = np.array([p[1] for p in PAIRS])
    xx = x[:, :, ii] * x[:, :, jj]                   # [B, R, 36]
    xxt = xx.transpose(1, 2, 0)                      # [R, 36, B]
    xxs = np.zeros((NCORES, NG, 128, B), np.float32)
    xxs[:, :, :108] = xxt.reshape(NCORES, NG, 108, B)
    xxs = np.ascontiguousarray(
        xxs.reshape(NCORES, NQ, 4, 128, B).transpose(0, 1, 3, 2, 4))

    # Gram columns: [cores, 48, 128, 30] block-diagonal over the 3 routes
    W64 = W0.astype(np.float64)
    G = np.einsum('croi,croj->crij', W64, W64)       # [C, R, I, I]
    Gsym = G[:, :, ii, jj] * np.where(ii == jj, 1.0, 2.0)   # [C, R, 36]
    Gt = Gsym.transpose(1, 2, 0).astype(np.float32)  # [R, 36, C]
    Gt = Gt.reshape(NCORES, NG, 3, NPAIR, C)
    gs = np.zeros((NCORES, NG, 128, 30), np.float32)
    for r in range(3):
        gs[:, :, r * NPAIR:(r + 1) * NPAIR, r * C:(r + 1) * C] = Gt[:, :, r]
    gs = np.ascontiguousarray(gs.transpose(0, 2, 1, 3))   # [cores, 128, 48, 30]
    return xs, wm, xxs, gs


def _np_dt(dt):
    if dt == BF16:
        import ml_dtypes
        return ml_dtypes.bfloat16
    if dt == F16:
        return np.float16
    return np.float32


def _in_maps(packed, mode: str = "full"):
    xs, wm, xxs, gs = packed
    u_dt, sq_dt, _ = _dtypes(mode)
    u_np, sq_np = _np_dt(u_dt), _np_dt(sq_dt)
    xs = xs.astype(u_np)
    wm = wm.astype(u_np)
    xxs = xxs.astype(sq_np)
    gs = gs.astype(sq_np)
    return [{"xs": xs[k], "wm": wm[k], "xxs": xxs[k], "gs": gs[k]}
            for k in range(NCORES)]


def _unpack_outputs(results):
    """Per-core out [2, NG, 128, 480] -> full [B, C, R, O]."""
    full = np.empty((B, C, R, O), dtype=np.float32)
    for k in range(NCORES):
        ok = np.asarray(results[k]["out"], dtype=np.float32)
        ok = ok.reshape(2, NG, 128, 3, C, O)
        # dims: h, g, p, r, c, o ; route_local = 3g + r
        fk = ok.transpose(0, 2, 4, 1, 3, 5).reshape(B, C, RL, O)
        full[:, :, k * RL:(k + 1) * RL, :] = fk
    return full


def run_packed(packed, reps: int = 1, mode: str = "full"):
    nc = _get_nc(reps, mode)
    return run_bass_kernel_spmd(nc, _in_maps(packed, mode),
                                list(range(NCORES)))


def kernel(x: np.ndarray, W: np.ndarray, **_ignored):
    x = np.asarray(x, dtype=np.float32)
    W = np.asarray(W, dtype=np.float32)
    assert x.shape == (B, R, I), x.shape
    packed = _pack_inputs(x, W)
    res = run_packed(packed)
    return _unpack_outputs(res.results)

